# revision 25
# baseline (speedup 1.0000x reference)
"""MemoryReader sparse-attention kernel for 8x TRN2 NeuronCores.

Math (exact restructuring of the reference):
  Each query q attends to exactly slots [64q, 64q+64) (block-diag SLOT_MASK,
  memory_mask all ones).  K/V projections are folded algebraically:
    logits[b,h,q,m] = qa[b,h,q,:] . memory[b,m,:] / 8
        with qa = ((queries+cond) @ qw^T + qb)_h @ kw_h   (kb drops: shift-inv)
    ctxv[b,h,q,:]  = sum_j w[b,h,q,j] memory[b,chunk_q(j),:]
    attn_h = ctxv_h @ vw_h^T + vb_h                       (sum w = 1)

Host-side folding: cond/pq/qa/gate/q_resid are tiny (O(B*Q*D)) and computed
in numpy; the device kernel only streams `memory` once per layout.

Device: per (batch, slot-group of 512): one 1MiB fp8 DMA delivering BOTH
layouts (slot-major for AV lhsT, d-major for QK rhs), QK+mask matmuls into
PSUM, mask-safe softmax (no max-subtract: logits are O(1), masked lanes are
-3750 -> exp==0), PE transpose of weights, AV matmuls, then per-batch head
projection + fused out_proj + LN + gate.

Compute dtype: fp8(e4m3) operands for all big matmuls, f32 PSUM accumulation,
f32 softmax/LN statistics. Sharding: data-parallel over batch B=16 -> 2 per
core, no collectives.
"""
import sys
for _p in ("/opt/trn_rl_repo", "/root/.axon_site/_ro/trn_rl_repo"):
    if _p not in sys.path:
        sys.path.append(_p)

import numpy as np
import ml_dtypes

B, M, D, Q, H = 16, 4096, 1024, 64, 16
HD = D // H
NCORES = 8
BL = B // NCORES          # batches per core
SG = 8                    # slot groups per batch (512 slots each)
SGS = M // SG             # 512
NEG = -30000.0
T = BL * Q                # 128 tokens per core

_cache = {}

FP8 = ml_dtypes.float8_e4m3
BF16 = ml_dtypes.bfloat16


def _build():
    import concourse.bass as bass
    import concourse.mybir as mybir
    from concourse import bacc
    from concourse.masks import make_identity
    from concourse.tile import TileContext

    dt = mybir.dt
    AF = mybir.ActivationFunctionType

    nc = bacc.Bacc("TRN2", target_bir_lowering=False, debug=False)

    # ---- DRAM I/O ----
    # AV-side memory (slot-major) in bf16; QK-side memory (d-major, x4) in fp8
    # only d-slabs 0-1 are uploaded pre-transposed; slabs 2-7 are transposed
    # on the PE from the bf16 AV copy (saves 17.5us of DMA per core)
    memav = nc.dram_tensor("memav", [BL * SG, 128, 4096], dt.bfloat16, kind="ExternalInput")
    memqk = nc.dram_tensor("memqk", [BL * SG, 128, 1024], dt.float8e4, kind="ExternalInput")
    qaT = nc.dram_tensor("qaT", [128, 8, 2 * D], dt.float8e4, kind="ExternalInput")
    vwT = nc.dram_tensor("vwT", [128, 8, D], dt.bfloat16, kind="ExternalInput")
    outwT = nc.dram_tensor("outwT", [128, 8, D], dt.bfloat16, kind="ExternalInput")
    vbp = nc.dram_tensor("vbp", [128, 8], dt.float32, kind="ExternalInput")
    q_resid_in = nc.dram_tensor("q_resid", [128, D], dt.bfloat16, kind="ExternalInput")
    gate_in = nc.dram_tensor("gate_t", [128, 1], dt.float32, kind="ExternalInput")
    lng_in = nc.dram_tensor("lng", [D], dt.bfloat16, kind="ExternalInput")
    lnb_in = nc.dram_tensor("lnb", [D], dt.bfloat16, kind="ExternalInput")
    maskL = nc.dram_tensor("maskL", [SG, 128], dt.bfloat16, kind="ExternalInput")
    maskR = nc.dram_tensor("maskR", [SG, SGS], dt.bfloat16, kind="ExternalInput")
    out = nc.dram_tensor("out", [T, D], dt.float32, kind="ExternalOutput")

    with TileContext(nc) as tc:
        import contextlib
        est = contextlib.ExitStack()
        persist = est.enter_context(tc.tile_pool(name="persist", bufs=1))
        mempool = est.enter_context(tc.tile_pool(name="mempool", bufs=4))
        wpool = est.enter_context(tc.tile_pool(name="wpool", bufs=3))
        psQK = est.enter_context(tc.tile_pool(name="psQK", bufs=3, space="PSUM"))
        psT = est.enter_context(tc.tile_pool(name="psT", bufs=3, space="PSUM"))
        psAV = est.enter_context(tc.tile_pool(name="psAV", bufs=2, space="PSUM"))

        # ---------- prologue: only what the first QK needs ----------
        # qaT split per batch so QK(0) waits only on the first quarter
        qaT_sb = [persist.tile([128, 8, D], dt.float8e4, tag=f"qaT{b}", name=f"qaT{b}")
                  for b in range(BL)]
        nc.sync.dma_start(out=qaT_sb[0][:, :, 0:512], in_=qaT[:, :, 0:512])
        mL = persist.tile([SG, 128], dt.bfloat16)
        nc.sync.dma_start(out=mL, in_=maskL[:, :])
        mR = persist.tile([SG, SGS], dt.bfloat16)
        nc.sync.dma_start(out=mR, in_=maskR[:, :])
        vb_sb = persist.tile([128, 8], dt.float32)
        nc.sync.dma_start(out=vb_sb, in_=vbp[:, :])
        gate_t = persist.tile([128, 1], dt.float32)
        nc.sync.dma_start(out=gate_t, in_=gate_in[:, :])

        ident = persist.tile([128, 128], dt.bfloat16)
        make_identity(nc, ident)
        eps_sb = persist.tile([128, 1], dt.float32)
        nc.vector.memset(eps_sb, 1e-5)
        # hoist ACT function-table loads off the critical path
        dummy = persist.tile([1, 1], dt.float32)
        nc.scalar.activation(out=dummy, in_=eps_sb[0:1, 0:1], func=AF.Exp)
        nc.scalar.activation(out=dummy, in_=eps_sb[0:1, 0:1], func=AF.Sqrt, bias=eps_sb[0:1, 0:1])

        # late persists (declared now, uploaded staggered inside the loop)
        vwT_sb = persist.tile([128, 8, D], dt.bfloat16)
        outwT_sb = persist.tile([128, 8, D], dt.bfloat16)
        q_resid = persist.tile([128, D], dt.bfloat16)
        lng_rep = persist.tile([128, D], dt.bfloat16)
        lnbg = persist.tile([128, D], dt.bfloat16)

        # ctxvT[p_d, dt, b, h, q] ; attnT[p=(hh,hd), rt, t=(b,q)]
        ctxvT = persist.tile([128, 8, BL, H, Q], dt.bfloat16)
        attnT = persist.tile([128, 8, T], dt.bfloat16)
        readout = persist.tile([128, D], dt.float32)

        NIT = BL * SG
        iters = [(b, sg) for b in range(BL) for sg in range(SG)]
        blks, blkTs, plogs = {}, {}, {}

        def emit_load(i):
            blk = mempool.tile([128, 4096], dt.bfloat16, tag="blk")
            nc.sync.dma_start(out=blk, in_=memav[i])
            memT = mempool.tile([128, 8, 512], dt.float8e4, tag="memT")
            nc.sync.dma_start(out=memT[:, 0:2, :], in_=memqk[i])
            blkTs[i], blks[i] = memT, blk

        def emit_trans(i):
            # transpose d-slabs 2-7 of this block on the PE (x4 to match memqk)
            blk = blks[i]
            memT = blkTs[i]
            for g in range(3):
                ptr = psT.tile([128, 2, 512], dt.bfloat16, tag="pwt", name=f"ptr{g}")
                for j in range(2):
                    ds = 2 + g * 2 + j
                    for cb in range(4):
                        nc.tensor.transpose(ptr[:, j, cb * 128:(cb + 1) * 128],
                                            blk[:, cb * 1024 + ds * 128:cb * 1024 + (ds + 1) * 128],
                                            ident)
                dst = memT[:, 2 + g * 2:4 + g * 2, :]
                if g == 2:
                    nc.vector.tensor_scalar_mul(dst, ptr, 4.0)
                else:
                    nc.scalar.activation(out=dst, in_=ptr, func=AF.Copy, scale=4.0)

        def emit_qk(i):
            b, sg = iters[i]
            memT = blkTs[i]
            plog = psQK.tile([128, SGS], dt.float32, tag="plog")
            for dp in range(4):
                # DoubleRow: contracts d-slabs 2dp and 2dp+1 in one pass
                nc.tensor.matmul(plog, qaT_sb[b][:, 2 * dp:2 * dp + 2, sg * 128:(sg + 1) * 128],
                                 memT[:, 2 * dp:2 * dp + 2, :], start=(dp == 0), stop=False,
                                 perf_mode=mybir.MatmulPerfMode.DoubleRow)
            nc.tensor.matmul(plog, mL, mR, start=False, stop=True)
            plogs[i] = plog

        wTs = {}

        def emit_softmax(i):
            # softmax over slots; logits carry x32 (qa x8, memT x4) so the
            # exp scale is 0.125/32.  No max-subtract needed: valid logits
            # are O(1), masked are -30000*0.125/32 -> exp underflows to 0
            plog = plogs.pop(i)
            w_sb = wpool.tile([128, SGS], dt.bfloat16, tag="w")
            wsum = wpool.tile([128, 1], dt.float32, tag="wsum")
            nc.scalar.activation(out=w_sb, in_=plog, func=AF.Exp, scale=0.125 / 32.0,
                                 accum_out=wsum)
            recip = wpool.tile([128, 1], dt.float32, tag="recip")
            nc.vector.reciprocal(out=recip, in_=wsum)
            # diag(recip): fuses the normalize into the transpose matmul
            diag = wpool.tile([128, 128], dt.bfloat16, tag="diag")
            nc.vector.tensor_scalar_mul(diag, ident, recip)

            # transpose-and-normalize w -> wT[p_s, cb, t] = w[t, s]*recip[t]
            pwt = psT.tile([128, 4, 128], dt.float32, tag="pwt")
            for cb in range(4):
                nc.tensor.matmul(pwt[:, cb, :], w_sb[:, cb * 128:(cb + 1) * 128], diag,
                                 start=True, stop=True)
            wT = wpool.tile([128, 4, 128], dt.bfloat16, tag="wT")
            nc.scalar.activation(out=wT, in_=pwt, func=AF.Copy)
            wTs[i] = wT

        def emit_avmm(i):
            b, sg = iters[i]
            wT = wTs.pop(i)
            # AV: ctxvT[d-slab, t] = sum_cb mem_cb^T @ wT_cb   (bf16)
            memv = blks.pop(i).rearrange("p (cb d) -> p cb d", cb=4)
            for dg in range(2):
                pav = psAV.tile([128, 512], dt.float32, tag="pav")
                for j in range(4):
                    ds = dg * 4 + j
                    for cb in range(4):
                        nc.tensor.matmul(pav[:, j * 128:(j + 1) * 128],
                                         memv[:, cb, ds * 128:(ds + 1) * 128],
                                         wT[:, cb, :], start=(cb == 0), stop=(cb == 3))
                dstv = ctxvT[:, dg * 4:(dg + 1) * 4, b, :, sg * SG:(sg + 1) * SG]
                src = pav.rearrange("p (j q h) -> p j h q", j=4, q=SG)
                if dg == 0:
                    nc.vector.tensor_copy(out=dstv, in_=src)
                else:
                    nc.scalar.activation(out=dstv, in_=src, func=AF.Copy)

        def emit_heads(b):
            # attn_h = ctxv_h @ vw_h^T + vb_h
            for rt in range(8):
                pat = psT.tile([128, Q], dt.float32, tag="pwt")
                for hh in range(2):
                    h = rt * 2 + hh
                    for dt8 in range(8):
                        nc.tensor.matmul(pat[hh * 64:(hh + 1) * 64, :],
                                         vwT_sb[:, dt8, h * HD:(h + 1) * HD],
                                         ctxvT[:, dt8, b, h, :],
                                         start=(dt8 == 0), stop=(dt8 == 7))
                nc.vector.tensor_scalar_add(attnT[:, rt, b * Q:(b + 1) * Q], pat,
                                            vb_sb[:, rt:rt + 1])

        def emit_late(i):
            # staggered parameter uploads, hidden behind early loop iterations
            if i == 0:
                nc.sync.dma_start(out=qaT_sb[0][:, :, 512:D], in_=qaT[:, :, 512:D])
            elif i == 1:
                nc.sync.dma_start(out=vwT_sb, in_=vwT[:, :, :])
            elif i == 2:
                nc.sync.dma_start(out=qaT_sb[1][:, :, 0:512], in_=qaT[:, :, D:D + 512])
                nc.sync.dma_start(out=q_resid, in_=q_resid_in[:, :])
            elif i == 3:
                nc.sync.dma_start(out=qaT_sb[1][:, :, 512:D], in_=qaT[:, :, D + 512:2 * D])
                nc.sync.dma_start(out=outwT_sb, in_=outwT[:, :, :])
            elif i == 4:
                nc.sync.dma_start(out=lng_rep, in_=lng_in.rearrange("(o d) -> o d", o=1).to_broadcast((128, D)))
                nc.sync.dma_start(out=lnbg, in_=lnb_in.rearrange("(o d) -> o d", o=1).to_broadcast((128, D)))
            elif i == 5:
                # lnbg = lnb * gate  (precomputed off the critical tail)
                nc.vector.tensor_scalar_mul(lnbg, lnbg, gate_t)

        # per-batch out_proj + residual + LN + gate + store; b=0's copy of this
        # overlaps the b=1 main loop, only b=1's runs in the tail
        stats = persist.tile([128, 2, 6], dt.float32)
        mv = persist.tile([128, 2], dt.float32)
        rstd = persist.tile([128, 1], dt.float32)
        rg = persist.tile([128, 1], dt.float32)
        final = persist.tile([128, D], dt.float32)

        def emit_finish(b):
            rows = slice(b * Q, (b + 1) * Q)
            for nh in range(2):
                po = psQK.tile([Q, 512], dt.float32, tag="plog", name=f"po{b}{nh}")
                for rt in range(8):
                    nc.tensor.matmul(po, attnT[:, rt, rows], outwT_sb[:, rt, nh * 512:(nh + 1) * 512],
                                     start=(rt == 0), stop=(rt == 7))
                nc.vector.tensor_add(out=readout[rows, nh * 512:(nh + 1) * 512], in0=po,
                                     in1=q_resid[rows, nh * 512:(nh + 1) * 512])
            for sub in range(2):
                nc.vector.bn_stats(out=stats[rows, sub, :], in_=readout[rows, sub * 512:(sub + 1) * 512])
            nc.vector.bn_aggr(out=mv[rows, :], in_=stats[rows, :, :])
            nc.scalar.activation(out=rstd[rows, :], in_=mv[rows, 1:2], func=AF.Sqrt,
                                 bias=eps_sb[rows, :], scale=1.0)
            nc.vector.reciprocal(out=rstd[rows, :], in_=rstd[rows, :])
            nc.vector.tensor_scalar_mul(rg[rows, :], rstd[rows, :], gate_t[rows, :])
            nc.vector.tensor_scalar(out=final[rows, :], in0=readout[rows, :],
                                    scalar1=mv[rows, 0:1], scalar2=rg[rows, :],
                                    op0=mybir.AluOpType.subtract, op1=mybir.AluOpType.mult)
            nc.vector.tensor_mul(out=final[rows, :], in0=final[rows, :], in1=lng_rep[rows, :])
            nc.vector.tensor_add(out=final[rows, :], in0=final[rows, :], in1=lnbg[rows, :])
            nc.sync.dma_start(out=out[rows, :], in_=final[rows, :])

        # ---------- software-pipelined main loop ----------
        emit_load(0)
        emit_load(1)
        emit_trans(0)
        emit_qk(0)
        for i in range(NIT):
            if i + 2 < NIT:
                emit_load(i + 2)
            emit_softmax(i)
            if i + 1 < NIT:
                emit_trans(i + 1)
            emit_avmm(i)
            if i + 1 < NIT:
                emit_qk(i + 1)
            emit_late(i)
            b, sg = iters[i]
            if sg == SG - 1:
                emit_heads(b)
                emit_finish(b)

        est.close()

    nc.compile()
    return nc


def _prep_host(inputs):
    x = {k: np.ascontiguousarray(np.asarray(v)) for k, v in inputs.items()}
    ipw = x["in_proj_w"].astype(np.float32)
    ipb = x["in_proj_b"].astype(np.float32)
    qw, kw, vw = ipw[:D], ipw[D:2 * D], ipw[2 * D:]
    qb, vb = ipb[:D], ipb[2 * D:]
    memory = x["memory"].astype(np.float32)
    context = x["context"].astype(np.float32)
    queries = x["queries"].astype(np.float32)

    # ---- host folding of the small projections ----
    cond = context @ x["ctx_w"].astype(np.float32).T + x["ctx_b"].astype(np.float32)  # [B, D]
    qt = queries[None, :, :] + cond[:, None, :]                                       # [B, Q, D]
    pq = qt @ qw.T + qb                                                               # [B, Q, D]
    pq_r = pq.reshape(B, Q, H, HD)
    kw_r = kw.reshape(H, HD, D)
    qa = np.einsum("bqhi,hid->bhqd", pq_r, kw_r, optimize=True)                       # [B, H, Q, D]
    gate = 1.0 / (1.0 + np.exp(-(context @ x["gate_w"].astype(np.float32).T
                                 + x["gate_b"].astype(np.float32))))                  # [B, Q]
    q_resid = 0.1 * qt + x["out_proj_b"].astype(np.float32)                           # [B, Q, D]

    # vwT[p_d, dt, (h,hd)] ; outwT[p=(hh,hd), rt, d_out]
    vwT = np.ascontiguousarray(vw.T.reshape(8, 128, D).transpose(1, 0, 2)).astype(BF16)
    ow_t = x["out_proj_w"].astype(np.float32).T                                       # [c=(h,hd), d_out]
    ow_r = ow_t.reshape(8, 2, 64, D)                                                  # [rt, hh, hd, d]
    outwT = np.ascontiguousarray(ow_r.transpose(1, 2, 0, 3).reshape(128, 8, D)).astype(BF16)
    vbp = np.ascontiguousarray(vb.reshape(8, 2, 64).transpose(1, 2, 0).reshape(128, 8))

    mLh = np.zeros((SG, 128), np.float32)
    for k in range(SG):
        mLh[k, k * 16:(k + 1) * 16] = 1.0
    mRh = np.full((SG, SGS), NEG, np.float32)
    for k in range(SG):
        mRh[k, k * 64:(k + 1) * 64] = 0.0

    shared = {
        "vwT": vwT,
        "outwT": outwT,
        "vbp": vbp,
        "lng": x["ln_g"].astype(BF16),
        "lnb": x["ln_b"].astype(BF16),
        "maskL": mLh.astype(BF16),
        "maskR": mRh.astype(BF16),
    }

    mem_bf = memory.astype(BF16)
    mem8 = (memory * 4.0).astype(FP8)   # x4: pushes values out of e4m3 subnormals

    in_maps = []
    for c in range(NCORES):
        im = dict(shared)
        bsl = slice(c * BL, (c + 1) * BL)
        # memav[it, p_s, cb*1024+d] bf16 ; memqk[it, p_d, dt*512+s] fp8 (x4)
        mc = mem_bf[bsl]                                      # [BL, 4096, 1024]
        im["memav"] = np.ascontiguousarray(
            mc.reshape(BL, SG, 4, 128, D).transpose(0, 1, 3, 2, 4)).reshape(BL * SG, 128, 4096)
        mc8 = mem8[bsl]
        im["memqk"] = np.ascontiguousarray(
            mc8.reshape(BL, SG, SGS, 8, 128).transpose(0, 1, 4, 3, 2)[:, :, :, 0:2, :]
        ).reshape(BL * SG, 128, 1024)

        # qaT[p_d, dt, t=(b,q,h)] = 8 * qa[b, h, q, dt*128+p_d]   (x8 scaling)
        qac = qa[bsl] * 8.0                                   # [BL, H, Q, D]
        qaT = qac.transpose(3, 0, 2, 1).reshape(8, 128, BL * Q * H).transpose(1, 0, 2)
        im["qaT"] = np.ascontiguousarray(qaT).astype(FP8)

        im["q_resid"] = np.ascontiguousarray(q_resid[bsl].reshape(T, D)).astype(BF16)
        im["gate_t"] = np.ascontiguousarray(gate[bsl].reshape(T, 1))
        in_maps.append(im)
    return in_maps


def kernel(**inputs):
    from concourse.bass_utils import run_bass_kernel_spmd
    if "nc" not in _cache:
        _cache["nc"] = _build()
    nc = _cache["nc"]
    in_maps = _prep_host(inputs)
    res = run_bass_kernel_spmd(nc, in_maps, list(range(NCORES)))
    _cache["last_result"] = res
    outs = [res.results[c]["out"].reshape(BL, Q, D) for c in range(NCORES)]
    return np.concatenate(outs, axis=0).astype(np.float32)


if __name__ == "__main__":
    d = np.load("/root/problem/ref_cache.npz")
    ins = {k: d[k] for k in d.files if k != "expected"}
    outv = kernel(**ins)
    err = np.abs(outv - d["expected"])
    print("absmax err", err.max(), "rel", err.max() / np.abs(d["expected"]).max())
